# revision 2
# baseline (speedup 1.0000x reference)
"""Multi-head attention (B=4, N=2048, C=1024, H=16, D=64) on 8 TRN2 NeuronCores.

Sharding: core c owns (batch b = c//2, sequence half = c%2) -> 1024 query
tokens, all 16 heads.  Each core computes K and V for its OWN half only;
the partner half arrives via pairwise AllGathers (replica groups [2b, 2b+1]).
Output is purely row-sharded -> host gather is a concat.

Performance notes (v2):
- The TRN2 activity monitor throttles the PE array to 50% utilization after
  a >~3.4us TensorE idle window, and a stream of <=51%-util attention matmuls
  can never un-trip it.  So the kernel is scheduled so TensorE never idles:
  K/V compute and their AllGathers are interleaved (K heads 0-3 -> AG, V
  tiles 0-3 -> AG, K 4-7 -> AG, V 4-7 -> AG) so all gathered data is SBUF
  resident before the QKV phase ends, and attention starts with zero gap.
- exp() is split between the Scalar engine (native Exp) and the Vector
  engine (Schraudolph bit-trick: bf16 = top half of f32, so
  p = bitcast_bf16(int16(S*scale*184.665 + 16250.5)) is one tensor_scalar);
  otherwise Scalar exp (285us) would be the wall.
- Softmax denominators come from a ones-column appended to V inside the PV
  matmul; normalization uses reciprocal_approx_fast (5x faster than the
  bit-exact reciprocal).
- Consolidated DMA: 5 big input loads, 4 stage DMAs, 8 unstage DMAs.
- All matmuls bf16 with f32 PSUM accumulate.
"""

import numpy as np
import ml_dtypes

import concourse.bass as bass
import concourse.mybir as mybir
import concourse.tile as tile
from concourse import bacc
from concourse.bass_utils import run_bass_kernel_spmd

B, N, C = 4, 2048, 1024
H, D = 16, 64
SCALE = D ** -0.5
NCORES = 8
NQ = N // 2          # query tokens per core (own half)
M = N                # key/value tokens after gather

BF16 = mybir.dt.bfloat16
F32 = mybir.dt.float32
I16 = mybir.dt.int16

# Schraudolph exp in bf16: exp(x*SCALE) ~= bitcast_bf16(int16(x*EXPA + EXPB))
EXPA = (2.0 ** 7 / np.log(2.0)) * SCALE
EXPB = 127.0 * 128.0 - 5.5
# which of the 16 m-tile iterations per head run exp on DVE instead of Scalar
DVE_EXP_IDX = {2, 5, 8, 11, 14}

_CACHE = {}
LAST_RESULTS = None


def _build():
    nc = bacc.Bacc(
        "TRN2",
        target_bir_lowering=False,
        debug=False,
        enable_asserts=False,
        num_devices=NCORES,
    )
    xoT = nc.dram_tensor("xoT", [C, NQ], BF16, kind="ExternalInput")
    wqkvT = nc.dram_tensor("wqkvT", [C, 3 * C], BF16, kind="ExternalInput")
    bqk = nc.dram_tensor("bqk", [128, 16], F32, kind="ExternalInput")
    bv = nc.dram_tensor("bv", [1, C], BF16, kind="ExternalInput")
    wprojT = nc.dram_tensor("wprojT", [C, C], BF16, kind="ExternalInput")
    bproj = nc.dram_tensor("bproj", [128, 8], F32, kind="ExternalInput")
    yT = nc.dram_tensor("yT", [C, NQ], F32, kind="ExternalOutput")

    groups = [[2 * b, 2 * b + 1] for b in range(B)]

    with tile.TileContext(nc) as tc:
        with (
            tc.tile_pool(name="persist", bufs=1) as pp,
            tc.tile_pool(name="psum", bufs=1, space="PSUM") as psp,
            tc.tile_pool(name="dram", bufs=1, space="DRAM") as dp,
        ):
            lp = tc.alloc_tile_pool(name="front", bufs=1)

            # ---- big input loads (sync queue: needed-first order) ----
            xo = lp.tile([128, 8, NQ], BF16, tag="xo", name="xo")
            nc.sync.dma_start(xo[:, :, :], xoT.rearrange("(c p) n -> p c n", p=128))
            wk = lp.tile([128, 8, C], BF16, tag="wk", name="wk")
            nc.sync.dma_start(
                wk[:, :, :],
                wqkvT.rearrange("(c p) o -> p c o", p=128)[:, :, C : 2 * C],
            )
            wv = lp.tile([128, 8, C], BF16, tag="wv", name="wv")
            nc.sync.dma_start(
                wv[:, :, :],
                wqkvT.rearrange("(c p) o -> p c o", p=128)[:, :, 2 * C : 3 * C],
            )
            wq = lp.tile([128, 8, C], BF16, tag="wq", name="wq")
            nc.sync.dma_start(
                wq[:, :, :],
                wqkvT.rearrange("(c p) o -> p c o", p=128)[:, :, 0:C],
            )

            # ---- small loads (scalar queue) ----
            bqk_sb = pp.tile([128, 16], F32, tag="bqk", name="bqk")
            nc.scalar.dma_start(bqk_sb[:, :], bqk[:, :])
            bv_sb = lp.tile([1, C], BF16, tag="bv", name="bv")
            nc.scalar.dma_start(bv_sb[:, :], bv[:, :])
            bp_sb = pp.tile([128, 8], F32, tag="bp", name="bp")
            nc.scalar.dma_start(bp_sb[:, :], bproj[:, :])

            bvb = lp.tile([128, C], BF16, tag="bvb", name="bvb")
            nc.gpsimd.partition_broadcast(bvb[:, :], bv_sb[:, :])

            # ---- persistent attention operands ----
            KT = pp.tile([128, 8, M], BF16, tag="KT", name="KT")
            QT = pp.tile([128, 8, NQ], BF16, tag="QT", name="QT")
            # V split in two chunks (j 0-3 / 4-7 of each half) so the PV
            # dependency clears as each gather chunk lands.
            Vb = [
                pp.tile([128, 2, 4, H, D + 1], BF16, tag=f"Vb{c}", name=f"Vb{c}")
                for c in range(2)
            ]
            A_sb = [
                pp.tile([128, NQ], BF16, tag=f"a{i}", name=f"a{i}") for i in range(8)
            ]

            # staging SBUF + DRAM bounce buffers
            kh = lp.tile([128, 8, NQ], BF16, tag="kh", name="kh")
            vh = lp.tile([128, 8, H, D + 1], BF16, tag="vh", name="vh")
            k_in = [dp.tile([512, NQ], BF16, tag=f"ki{c}", name=f"ki{c}") for c in range(2)]
            k_out = [
                dp.tile([2, 512, NQ], BF16, tag=f"ko{c}", name=f"ko{c}") for c in range(2)
            ]
            v_in = [
                dp.tile([512, H * (D + 1)], BF16, tag=f"vi{c}", name=f"vi{c}")
                for c in range(2)
            ]
            v_out = [
                dp.tile([2, 512, H * (D + 1)], BF16, tag=f"vo{c}", name=f"vo{c}")
                for c in range(2)
            ]

            def k_heads(c):
                # K output channels i*128..(i+1)*128 for own tokens, head
                # pairs i in chunk c; bias fused into the PSUM->SBUF copy.
                for i in range(4 * c, 4 * c + 4):
                    ps = psp.tile([128, NQ], F32, tag="mm", bufs=2, name="psk")
                    for ct in range(8):
                        for nch in range(2):
                            nc.tensor.matmul(
                                ps[:, nch * 512 : (nch + 1) * 512],
                                wk[:, ct, i * 128 : (i + 1) * 128],
                                xo[:, ct, nch * 512 : (nch + 1) * 512],
                                start=(ct == 0),
                                stop=(ct == 7),
                            )
                    nc.vector.tensor_scalar_add(
                        kh[:, i, :], ps[:, :], bqk_sb[:, 8 + i : 9 + i]
                    )
                nc.sync.dma_start(
                    k_in[c].rearrange("(i p) m -> p i m", p=128),
                    kh[:, 4 * c : 4 * c + 4, :],
                )
                nc.gpsimd.collective_compute(
                    "AllGather",
                    mybir.AluOpType.bypass,
                    replica_groups=groups,
                    ins=[k_in[c].opt()],
                    outs=[k_out[c].opt()],
                )

            def v_tiles(c):
                # V for own token tiles j in chunk c (all 16 heads); ones
                # column at d=D drives the softmax denominator in PV.
                for j in range(4 * c, 4 * c + 4):
                    ps = psp.tile([128, H, D], F32, tag="mm", bufs=2, name="psv")
                    for ct in range(8):
                        for vch in range(2):
                            nc.tensor.matmul(
                                ps[:, vch * 8 : (vch + 1) * 8, :],
                                xo[:, ct, j * 128 : (j + 1) * 128],
                                wv[:, ct, vch * 512 : (vch + 1) * 512],
                                start=(ct == 0),
                                stop=(ct == 7),
                            )
                    nc.vector.memset(vh[:, j, :, D : D + 1], 1.0)
                    nc.vector.tensor_tensor(
                        vh[:, j, :, 0:D], ps[:, :, :],
                        bvb[:, :].rearrange("p (h e) -> p h e", e=D),
                        op=mybir.AluOpType.add,
                    )
                nc.sync.dma_start(
                    v_in[c].rearrange("(j p) f -> p j f", p=128),
                    vh[:, 4 * c : 4 * c + 4, :, :].rearrange("p j h e -> p j (h e)"),
                )
                nc.gpsimd.collective_compute(
                    "AllGather",
                    mybir.AluOpType.bypass,
                    replica_groups=groups,
                    ins=[v_in[c].opt()],
                    outs=[v_out[c].opt()],
                )

            # interleave compute with the 4 collectives so each gather's
            # input is staged as early as possible
            k_heads(0)
            v_tiles(0)
            k_heads(1)
            v_tiles(1)

            # ---- unstage gathered K/V (scalar queue; both halves -> the
            # program stays identical on both cores of a pair) ----
            for c in range(2):
                for r in range(2):
                    nc.scalar.dma_start(
                        KT[:, 4 * c : 4 * c + 4, r * NQ : (r + 1) * NQ],
                        k_out[c][r].rearrange("(i p) m -> p i m", p=128),
                    )
            for c in range(2):
                for r in range(2):
                    nc.scalar.dma_start(
                        Vb[c][:, r, :, :, :].rearrange("p j h e -> p j (h e)"),
                        v_out[c][r].rearrange("(j p) f -> p j f", p=128),
                    )

            # ---- Q (own tokens; bias fused in copy) ----
            for i in range(8):
                ps = psp.tile([128, NQ], F32, tag="mm", bufs=2, name="psq")
                for ct in range(8):
                    for nch in range(2):
                        nc.tensor.matmul(
                            ps[:, nch * 512 : (nch + 1) * 512],
                            wq[:, ct, i * 128 : (i + 1) * 128],
                            xo[:, ct, nch * 512 : (nch + 1) * 512],
                            start=(ct == 0),
                            stop=(ct == 7),
                        )
                nc.vector.tensor_scalar_add(
                    QT[:, i, :], ps[:, :], bqk_sb[:, i : i + 1]
                )

            lp.release()
            wk2 = tc.alloc_tile_pool(name="attnwork", bufs=1)
            wp_sb = wk2.tile([128, 8, C], BF16, tag="wp", name="wp")
            nc.scalar.dma_start(
                wp_sb[:, :, :], wprojT.rearrange("(c p) o -> p c o", p=128)
            )

            # ---- attention ----
            # scores computed transposed (S^T[m, n]); PV accumulates over all
            # 16 m-tiles per head into one PSUM pair; denominators = row D of
            # the PV output via the ones-column of V.
            for h in range(H):
                i, poff = h // 2, (h % 2) * 64
                pv = [
                    psp.tile([65, 512], F32, tag=f"acc{j}", bufs=2, name=f"pv{j}")
                    for j in range(2)
                ]
                for mt in range(16):
                    r, j = mt // 8, mt % 8
                    vc, vj = j // 4, j % 4
                    sp = psp.tile([128, NQ], F32, tag="mm", bufs=2, name="pss")
                    for nch in range(2):
                        nc.tensor.matmul(
                            sp[:, nch * 512 : (nch + 1) * 512],
                            KT[poff : poff + 64, i, mt * 128 : (mt + 1) * 128],
                            QT[poff : poff + 64, i, nch * 512 : (nch + 1) * 512],
                            start=True,
                            stop=True,
                        )
                    p = wk2.tile([128, NQ], BF16, tag="p", bufs=4, name="p")
                    if mt in DVE_EXP_IDX:
                        nc.vector.tensor_scalar(
                            p[:, :].bitcast(I16), sp[:, :],
                            EXPA, EXPB,
                            op0=mybir.AluOpType.mult,
                            op1=mybir.AluOpType.add,
                        )
                    else:
                        nc.scalar.activation(
                            p[:, :], sp[:, :],
                            mybir.ActivationFunctionType.Exp, scale=SCALE,
                        )
                    for nch in range(2):
                        nc.tensor.matmul(
                            pv[nch][:, :],
                            Vb[vc][:, r, vj, h, :],
                            p[:, nch * 512 : (nch + 1) * 512],
                            start=(mt == 0),
                            stop=(mt == 15),
                            skip_group_check=True,
                        )
                # normalize: A[h] = PV[0:64] / PV[64]
                stage = wk2.tile([65, NQ], BF16, tag="st", bufs=3, name="stage")
                den = wk2.tile([1, NQ], F32, tag="den", bufs=2, name="den")
                for nch in range(2):
                    nc.vector.tensor_copy(
                        stage[:, nch * 512 : (nch + 1) * 512], pv[nch][:, :]
                    )
                    nc.vector.tensor_copy(
                        den[:, nch * 512 : (nch + 1) * 512], pv[nch][64:65, :]
                    )
                rcp = wk2.tile([1, NQ], F32, tag="rcp", bufs=2, name="rcp")
                nc.vector.reciprocal_approx_fast(rcp[:, :], den[:, :])
                rb = wk2.tile([64, NQ], F32, tag="rb", bufs=2, name="rb")
                nc.gpsimd.partition_broadcast(rb[:, :], rcp[:, :])
                nc.vector.tensor_mul(
                    A_sb[i][poff : poff + 64, :], stage[0:64, :], rb[:, :]
                )

            # ---- output projection (ot pairs: 4 open accumulators) ----
            for op2 in range(4):
                pss = [
                    psp.tile([128, 512], F32, tag=f"acc{nch}", bufs=2, name="psp")
                    for j in range(2)
                    for nch in range(2)
                ]
                for dd in range(8):
                    for j in range(2):
                        ot = op2 * 2 + j
                        for nch in range(2):
                            nc.tensor.matmul(
                                pss[j * 2 + nch][:, :],
                                wp_sb[:, dd, ot * 128 : (ot + 1) * 128],
                                A_sb[dd][:, nch * 512 : (nch + 1) * 512],
                                start=(dd == 0),
                                stop=(dd == 7),
                            )
                for j in range(2):
                    ot = op2 * 2 + j
                    for nch in range(2):
                        y = wk2.tile([128, 512], F32, tag="y", bufs=3, name="y")
                        nc.vector.tensor_scalar_add(
                            y[:, :], pss[j * 2 + nch][:, :],
                            bp_sb[:, ot : ot + 1],
                        )
                        nc.sync.dma_start(
                            yT[ot * 128 : (ot + 1) * 128, nch * 512 : (nch + 1) * 512],
                            y[:, :],
                        )
            wk2.release()

    nc.compile()
    return nc


def kernel(x, w_qkv, b_qkv, w_proj, b_proj):
    global LAST_RESULTS
    bf = ml_dtypes.bfloat16
    x = np.asarray(x, np.float32)
    w_qkv = np.asarray(w_qkv, np.float32)
    b_qkv = np.asarray(b_qkv, np.float32)
    w_proj = np.asarray(w_proj, np.float32)
    b_proj = np.asarray(b_proj, np.float32)

    wqkvT = np.ascontiguousarray(w_qkv.T.astype(bf))            # [1024, 3072]
    wprojT = np.ascontiguousarray(w_proj.T.astype(bf))          # [1024, 1024]
    bqk = np.ascontiguousarray(
        b_qkv[: 2 * C].reshape(16, 128).T.astype(np.float32)
    )                                                           # [128, 16]
    bv = np.ascontiguousarray(b_qkv[None, 2 * C :].astype(bf))  # [1, 1024]
    bproj = np.ascontiguousarray(
        b_proj.reshape(8, 128).T.astype(np.float32)
    )                                                           # [128, 8]

    in_maps = []
    for core in range(NCORES):
        b, half = core // 2, core % 2
        own = x[b][half * NQ : (half + 1) * NQ]                 # [1024, 1024]
        in_maps.append(
            {
                "xoT": np.ascontiguousarray(own.T.astype(bf)),
                "wqkvT": wqkvT,
                "bqk": bqk,
                "bv": bv,
                "wprojT": wprojT,
                "bproj": bproj,
            }
        )

    if "nc" not in _CACHE:
        _CACHE["nc"] = _build()
    nc = _CACHE["nc"]

    res = run_bass_kernel_spmd(nc, in_maps, core_ids=list(range(NCORES)))
    LAST_RESULTS = res

    out = np.empty((B, N, C), np.float32)
    for core in range(NCORES):
        b, half = core // 2, core % 2
        out[b, half * NQ : (half + 1) * NQ, :] = res.results[core]["yT"].T
    return out


if __name__ == "__main__":
    rng = np.random.default_rng(0)
    s = C ** -0.5
    ins = {
        "x": rng.standard_normal((B, N, C)).astype(np.float32),
        "w_qkv": (rng.standard_normal((3 * C, C)) * s).astype(np.float32),
        "b_qkv": (rng.standard_normal(3 * C) * 0.02).astype(np.float32),
        "w_proj": (rng.standard_normal((C, C)) * s).astype(np.float32),
        "b_proj": (rng.standard_normal(C) * 0.02).astype(np.float32),
    }
    y = kernel(**ins)
    print("out", y.shape, y.dtype, float(np.abs(y).mean()))
